# revision 18
# baseline (speedup 1.0000x reference)
"""Multi-head attention (B=4, C=1024, H=1, W=1500, 16 heads) on 8 TRN2 cores.

Sharding: core = 2*b + g  (b = batch 0..3, g = head-group 0..1).
Each core computes 8 heads (512 channels) for one batch and a partial
output projection; the two partials per batch are summed on the host
(plus the constant Wo@bv + bo term, which is folded out on the host).

Device kernel (per core, all fp32 data, fp32r matmuls):
  phase 1: q = (s*Wq_g) @ x + s*bq_g  [512,1500]  (s = d^-0.5 folded on host)
           k = Wk_g @ x               [512,1500]
           vT = x^T @ Wv_g^T          [1500,512] (+ ones column for softmax sums)
  phase 2: per head h: S^T[k,q] = k_h^T-slices x q_h  (no max subtraction;
           scores ~ N(0,1), exp shifted by -4 for safety)
           P^T = exp(S^T - 4); o = vT^T @ P^T with a ones-column matmul
           producing the softmax denominators on the complementary
           partition half; a K=1 ones matmul broadcasts the sums row back
           onto o's partitions; DVE reciprocal + multiply normalizes
           during PSUM evacuation.
  phase 3: out_partial = Wo_g^T-slices @ o  [1024,1500]
"""
import numpy as np

import concourse.bass as bass
import concourse.mybir as mybir
import concourse.tile as tile
from concourse import bacc
from concourse.bass_utils import run_bass_kernel_spmd
from contextlib import ExitStack

N_CORES = 8
B, C, W = 4, 1024, 1500
M = 512          # channels per core
NH = 8           # heads per core
D = 64           # head dim
NKB = (W + 127) // 128   # 12 key blocks
CHUNKS = [(0, 512), (512, 512), (1024, W - 1024)]
F32 = mybir.dt.float32
F32R = mybir.dt.float32r
AF = mybir.ActivationFunctionType
EXP_SHIFT = -4.0


def f32r(ap):
    return ap.bitcast(F32R)


def build_nc(nheads=NH, do_exp=True, phase3=True):
    nc = bacc.Bacc("TRN2", target_bir_lowering=False, debug=False)
    x_d = nc.dram_tensor("x", [C, W], F32, kind="ExternalInput").ap()
    wqT_d = nc.dram_tensor("wqT", [C, M], F32, kind="ExternalInput").ap()
    wkT_d = nc.dram_tensor("wkT", [C, M], F32, kind="ExternalInput").ap()
    wvT_d = nc.dram_tensor("wvT", [C, M], F32, kind="ExternalInput").ap()
    woT_d = nc.dram_tensor("woT", [M, C], F32, kind="ExternalInput").ap()
    bq_d = nc.dram_tensor("bq", [M, 1], F32, kind="ExternalInput").ap()
    ones_d = nc.dram_tensor("ones", [128, 64], F32, kind="ExternalInput").ap()
    out_d = nc.dram_tensor("out", [C, W], F32, kind="ExternalOutput").ap()

    with tile.TileContext(nc) as tc, ExitStack() as top:
        pp = top.enter_context(tc.tile_pool(name="persist", bufs=1))
        q_tiles = [pp.tile([128, W], F32, tag=f"q{i}", name=f"q{i}") for i in range(4)]
        k_tiles = [pp.tile([128, W], F32, tag=f"k{i}", name=f"k{i}") for i in range(4)]
        # vT tiles: cols 0..511 = v^T block, col 512 = ones (softmax sums)
        vT_tiles = [pp.tile([128, M + 1], F32, tag=f"vt{i}", name=f"vt{i}")
                    for i in range(NKB)]
        o_tiles = [pp.tile([128, W], F32, tag=f"o{i}", name=f"o{i}") for i in range(4)]
        ones_bc = pp.tile([128, 64], F32, tag="onesbc", name="ones_bc")
        bq_tiles = [pp.tile([128, 1], F32, tag=f"bq{i}", name=f"bq{i}")
                    for i in range(4)]
        shift_t = pp.tile([128, 1], F32, tag="shift", name="shift_t")
        nc.sync.dma_start(ones_bc[:], ones_d[:].bitcast(F32R))
        nc.vector.memset(shift_t[:], EXP_SHIFT)
        for i in range(4):
            nc.sync.dma_start(bq_tiles[i][:], bq_d[i * 128:(i + 1) * 128, :])

        # ---------------- Phase 1: QKV projections ----------------
        with ExitStack() as ph1:
            xp = ph1.enter_context(tc.tile_pool(name="xp", bufs=1))
            wp = ph1.enter_context(tc.tile_pool(name="wp", bufs=1))
            pj = ph1.enter_context(tc.tile_pool(name="pj", bufs=2, space="PSUM"))
            x_tiles = [xp.tile([128, W], F32, tag=f"x{i}", name=f"x{i}")
                       for i in range(8)]
            for i in range(8):
                nc.sync.dma_start(x_tiles[i][:], x_d[i * 128:(i + 1) * 128, :].bitcast(F32R))

            # vT = x^T @ WvT  (w-tile m-major), plus ones column
            wvs = [wp.tile([128, M], F32, tag=f"w{cb}", name=f"wv{cb}")
                   for cb in range(8)]
            for cb in range(8):
                nc.sync.dma_start(wvs[cb][:], wvT_d[cb * 128:(cb + 1) * 128, :].bitcast(F32R))
            for wt in range(NKB):
                wlen = min(128, W - wt * 128)
                ps = pj.tile([128, 512], F32, tag="pj0", name=f"pjv{wt}")
                for cb in range(8):
                    nc.tensor.matmul(
                        ps[:wlen, :],
                        lhsT=f32r(x_tiles[cb][:, wt * 128:wt * 128 + wlen]),
                        rhs=f32r(wvs[cb][:]),
                        start=(cb == 0), stop=(cb == 7))
                vt3 = vT_tiles[wt].rearrange("p (h c) -> p h c", c=65)
                nc.vector.tensor_copy(vt3[:wlen, :, 0:64], ps[:wlen, :])
                nc.sync.dma_start(vt3[:wlen, :, 64:65],
                                  ones_d[0:wlen, 0:8].bitcast(F32R))

            for w_d, dst, bias in ((wqT_d, q_tiles, bq_tiles),
                                   (wkT_d, k_tiles, None)):
                wts = [wp.tile([128, M], F32, tag=f"w{cb}", name=f"w{cb}")
                       for cb in range(8)]
                for cb in range(8):
                    nc.sync.dma_start(wts[cb][:], w_d[cb * 128:(cb + 1) * 128, :].bitcast(F32R))
                for mt in range(4):
                for wts, dst, bias in ((wqs, q_tiles, bq_tiles),
                                       (wks, k_tiles, None)):
                    ps = pj.tile([128, 1536], F32, tag="pjqk",
                                 name=f"pjqk{mt}")
                    for cb in range(8):
                        for c, (q0, qn) in enumerate(CHUNKS):
                            nc.tensor.matmul(
                                ps[:, c * 512:c * 512 + qn],
                                lhsT=wts[cb][:, mt * 128:(mt + 1) * 128],
                                rhs=x_tiles[cb][:, q0:q0 + qn],
                                start=(cb == 0), stop=(cb == 7))
                    if bias is not None:
                        nc.scalar.activation(dst[mt][:, 0:W], ps[:, 0:W],
                                             AF.Identity, bias=bias[mt][:])
                    else:
                        nc.vector.tensor_copy(dst[mt][:, 0:W], ps[:, 0:W])

        # ---------------- Phase 2: attention per head ----------------
        with ExitStack() as ph2:
            ptp = ph2.enter_context(tc.tile_pool(name="ptp", bufs=13))
            srp = ph2.enter_context(tc.tile_pool(name="srp", bufs=2))
            rcp = ph2.enter_context(tc.tile_pool(name="rcp", bufs=3))
            stp = ph2.enter_context(tc.tile_pool(name="stp", bufs=2, space="PSUM"))
            opp = ph2.enter_context(tc.tile_pool(name="opp", bufs=2, space="PSUM"))

            ohsp = ph2.enter_context(tc.tile_pool(name="ohsp", bufs=2))
            for h in range(nheads):
                ti, prow = h // 2, (h % 2) * 64
                o_ps = [opp.tile([128, 512], F32, tag=f"op{c}", name=f"op{h}_{c}")
                        for c in range(2)]
                pts = {}

                def emit_sc(kb):
                    klen = min(128, W - kb * 128)
                    pt = ptp.tile([128, W], F32R, tag="pt", name=f"pt{h}_{kb}")
                    pts[kb] = pt
                    st = stp.tile([128, 1536], F32, tag="st",
                                  name=f"st{h}_{kb}")
                    for c, (q0, qn) in enumerate(CHUNKS):
                        nc.tensor.matmul(
                            st[:klen, c * 512:c * 512 + qn],
                            lhsT=k_tiles[ti][prow:prow + 64,
                                             kb * 128:kb * 128 + klen],
                            rhs=q_tiles[ti][prow:prow + 64, q0:q0 + qn],
                            start=True, stop=True)
                    nc.scalar.activation(pt[:klen, 0:W], st[:klen, 0:W],
                                         AF.Exp, bias=shift_t[:klen, :])

                def emit_pv(kb):
                    klen = min(128, W - kb * 128)
                    for c in (0, 1):
                        q0, qn = CHUNKS[c]
                        nc.tensor.matmul(
                            o_ps[c][0:65, :qn],
                            lhsT=vT_tiles[kb][:klen, h * 65:h * 65 + 65],
                            rhs=pts[kb][:klen, q0:q0 + qn],
                            start=(kb == 0), stop=(kb == NKB - 1))

                for kb in range(NKB):
                    emit_sc(kb)
                    if kb >= 1:
                        emit_pv(kb - 1)
                emit_pv(NKB - 1)
                # pass B: PV chunk 2 accumulates into the third bank of the
                # last st tile (dead after its exp) - no new st allocation,
                # so the st tag keeps true double-buffering.
                op2 = st_last[0:128, 1024:1536]
                q0, qn = CHUNKS[2]
                for kb in range(NKB):
                    klen = min(128, W - kb * 128)
                    nc.tensor.matmul(
                        op2[0:65, :qn],
                        lhsT=vT_tiles[kb][:klen, h * 65:h * 65 + 65],
                        rhs=pts[kb][:klen, q0:q0 + qn],
                        start=(kb == 0), stop=(kb == NKB - 1))
                o_ps.append(op2)
                # normalization
                sr = srp.tile([128, W], F32R, tag="sr", name=f"sr{h}")
                if prow == 0:
                    dst_tile = o_tiles[ti]
                else:
                    dst_tile = ohsp.tile([128, W], F32R, tag="ohs",
                                         name=f"ohs{h}")
                for c, (q0, qn) in enumerate(CHUNKS):
                    nc.vector.tensor_copy(sr[64:65, q0:q0 + qn],
                                          o_ps[c][64:65, :qn])
                    bc = stp.tile([128, 512], F32, tag="st",
                                  name=f"bc{h}_{c}")
                    nc.tensor.matmul(
                        bc[0:64, :qn],
                        lhsT=ones_bc[64:65, :],
                        rhs=sr[64:65, q0:q0 + qn],
                        start=True, stop=True)
                    rc = rcp.tile([128, 512], F32, tag="rc",
                                  name=f"rc{h}_{c}")
                    nc.vector.reciprocal_approx_fast(rc[0:64, :qn],
                                                     bc[0:64, :qn])
                    nc.vector.tensor_mul(
                        dst_tile[0:64, q0:q0 + qn],
                        o_ps[c][0:64, :qn], rc[0:64, :qn])
                if prow != 0:
                    nc.sync.dma_start(o_tiles[ti][64:128, :],
                                      dst_tile[0:64, :])

        # ---------------- Phase 3: output projection ----------------
        with ExitStack() as ph3:
            wop = ph3.enter_context(tc.tile_pool(name="wop", bufs=1))
            osp = ph3.enter_context(tc.tile_pool(name="osp", bufs=2))
            oup = ph3.enter_context(tc.tile_pool(name="oup", bufs=2, space="PSUM"))
            woT_tiles = [wop.tile([128, C], F32, tag=f"wo{i}", name=f"wo{i}")
                         for i in range(4)]
            for i in range(4):
                nc.sync.dma_start(woT_tiles[i][:], woT_d[i * 128:(i + 1) * 128, :].bitcast(F32R))
            for mt in range(8 if phase3 else 0):
                ps = [oup.tile([128, 512], F32, tag=f"ou{c}", name=f"ou{mt}_{c}")
                      for c in range(3)]
                for kb in range(4):
                    for c, (q0, qn) in enumerate(CHUNKS):
                        nc.tensor.matmul(
                            ps[c][:, :qn],
                            lhsT=f32r(woT_tiles[kb][:, mt * 128:(mt + 1) * 128]),
                            rhs=f32r(o_tiles[kb][:, q0:q0 + qn]),
                            start=(kb == 0), stop=(kb == 3))
                ost = osp.tile([128, W], F32, tag="ost", name=f"ost{mt}")
                for c, (q0, qn) in enumerate(CHUNKS):
                    nc.scalar.copy(ost[:, q0:q0 + qn], ps[c][:, :qn])
                nc.sync.dma_start(out_d[mt * 128:(mt + 1) * 128, :], ost[:])

    nc.compile()
    return nc


_NC = None


def get_nc():
    global _NC
    if _NC is None:
        _NC = build_nc()
    return _NC


def make_in_maps(x, Wq, bq, Wk, Wv, Wo):
    s = np.float32((C // 16) ** -0.5)  # d^-0.5 = 0.125
    x = np.asarray(x, np.float32)
    Wq = np.asarray(Wq, np.float32)
    Wk = np.asarray(Wk, np.float32)
    Wv = np.asarray(Wv, np.float32)
    Wo = np.asarray(Wo, np.float32)
    bq = np.asarray(bq, np.float32)
    in_maps = []
    for core in range(N_CORES):
        b, g = core // 2, core % 2
        rs = slice(g * M, (g + 1) * M)
        in_maps.append({
            "x": np.ascontiguousarray(x[b, :, 0, :]),
            "wqT": np.ascontiguousarray((Wq[rs] * s).T),
            "wkT": np.ascontiguousarray(Wk[rs].T),
            "wvT": np.ascontiguousarray(Wv[rs].T),
            "woT": np.ascontiguousarray(Wo[:, rs].T),
            "bq": np.ascontiguousarray((bq[rs] * s).reshape(M, 1)),
            "ones": np.ones((128, 64), np.float32),
        })
    return in_maps


def assemble(results, Wo, bv, bo):
    Wo = np.asarray(Wo, np.float32)
    bv = np.asarray(bv, np.float32)
    bo = np.asarray(bo, np.float32)
    const = (Wo @ bv + bo).astype(np.float32)[:, None]
    out = np.empty((B, C, 1, W), np.float32)
    for b in range(B):
        out[b, :, 0, :] = (results[2 * b]["out"] + results[2 * b + 1]["out"]
                           + const)
    return out


def kernel(x, Wq, bq, Wk, Wv, bv, Wo, bo):
    nc = get_nc()
    in_maps = make_in_maps(x, Wq, bq, Wk, Wv, Wo)
    res = run_bass_kernel_spmd(nc, in_maps, core_ids=list(range(N_CORES)))
    return assemble(res.results, Wo, bv, bo)
